# revision 42
# baseline (speedup 1.0000x reference)
"""Trainium2 Bass kernel for nn_DKAModule (dynamic-kernel attention), v3.

Data-parallel over B*n = 8192 tokens -> TPC=1024 per core (+10-token halo).
All matmuls bf16 (1 cycle/col on PE at 2.4GHz when back-to-back).

Per core, software-pipelined over heads (group g runs stage1 of head g,
band matmuls of head g-1, diag/chain tail of head g-2):

  stage1:  xp_m = W_in-block^T @ x^T + b_in     (PE; Act evac to bf16)
  band:    xtd  = 10-tile DMA transpose of xp_h (1 trigger, SP)
           ps_s = per-128-token-tile banded-conv matmuls (PE: C + L/R
                  halo slices, accumulated in PSUM). The per-token
                  coefficients c (= alpha * x_proj_h @ Wc) are folded
                  into the band matrices ON HOST, so ps_s = S*c already.
           cs   = plain PSUM evac (Act copy, bf16)
  tail:    o_h  = sum_r diag(V_r) @ cs_r        (PE, vdiag)
                + static conv:  k=11 heads via diag matmuls (PE, gdiag)
                                k=3,7,21 heads via DVE STT MAC chains
                  (chains <= 7 long for bf16 accumulation error)
  stage4:  out  = o^T-blocks @ W_out^T (+ b_out) (PE; Act/DVE evac)
"""
import sys
import types

import ml_dtypes
import numpy as np

BF16 = ml_dtypes.bfloat16

KS = [3, 3, 7, 7, 11, 11, 21, 21]
H, DM, DH, R, B, N = 8, 1024, 128, 4, 2, 4096
NC = 8
TPC = B * N // NC
PAD = 10
LP = 128  # left zero-pad columns in xp
XF = 1280  # padded xp width = 10 transpose tiles
NT = TPC // 128  # 8 token tiles
TH = TPC + 2 * PAD  # 1044 valid x columns
HEADS = (6, 7, 4, 5, 2, 3, 1, 0)  # k=21 first (long DVE chains), k=3 last
PE_STATIC_HEADS = (4, 5)  # k=11 static conv via PE diag matmuls
S1CH = [(0, 512), (512, 512), (1024, 20)]

_MODULE_CACHE = {}


def _install_ntff_hook_shim():
    """This image's antenv lacks axon_hooks; provide it so profiling works."""
    if "antenv.axon_hooks" in sys.modules:
        return
    try:
        from trn_agent_boot.trn_boot import _ntff_profile_via_ctypes

        hook = _ntff_profile_via_ctypes("/opt/axon/libaxon_pjrt.so")
    except Exception:
        hook = None
    mod = types.ModuleType("antenv.axon_hooks")
    mod.get_axon_ntff_profile_hook = lambda: hook
    mod.set_axon_ntff_profile_hook = lambda h: None
    sys.modules["antenv.axon_hooks"] = mod


def _split_multi_waits(nc, mybir):
    """walrus codegen allows a single sync-wait per instruction; hoist
    extras onto a chain of single-wait NoOps on the same engine."""
    for f in nc.m.functions:
        for blk in f.blocks:
            new_insts = []
            for inst in blk.instructions:
                si = getattr(inst, "sync_info", None)
                ow = list(si.on_wait) if si and si.on_wait else []
                if len(ow) >= 2:
                    for i, w in enumerate(ow[:-1]):
                        new_insts.append(
                            mybir.InstNoOp(
                                name=f"{inst.name}-wn{i}",
                                ins=[],
                                outs=[],
                                engine=inst.engine,
                                sync_info=mybir.SyncInfo(on_wait=[w], on_update=[]),
                            )
                        )
                    inst.sync_info = mybir.SyncInfo(
                        on_wait=[ow[-1]],
                        on_update=list(si.on_update) if si.on_update else [],
                    )
                new_insts.append(inst)
            blk.instructions = new_insts


def _tile_cols(h):
    """Band cols per token-tile for head h: C (R*128) + L (R*p) + R (R*p)."""
    p = KS[h] // 2
    return R * (128 + 2 * p)


def _band_off(h):
    """Column offset of head h's packed per-tile band blocks."""
    off = 0
    for g in range(h):
        off += NT * _tile_cols(g)
    return off


BAND_TOTAL = _band_off(H - 1) + NT * _tile_cols(H - 1)

# static taps handled on PE via gdiag matmuls (rest go to DVE chains)
PE_TAPS = {4: 11, 5: 11}
GD_OFF = {}
_o = 0
for _h in sorted(PE_TAPS):
    GD_OFF[_h] = _o
    _o += PE_TAPS[_h] * DH
GD_TOTAL = _o


def _build_module(has_bias):
    import concourse.bass as bass
    import concourse.tile as tile
    from concourse import mybir

    f32 = mybir.dt.float32
    bf16 = mybir.dt.bfloat16
    MULT = mybir.AluOpType.mult
    ADD = mybir.AluOpType.add
    IDENT = mybir.ActivationFunctionType.Identity

    nc = bass.Bass(trn_type="TRN2")

    xT_d = nc.dram_tensor("xT", [DM, TH], bf16, kind="ExternalInput")
    w_inT_d = nc.dram_tensor("w_inT", [DM, DM], bf16, kind="ExternalInput")
    w_outT_d = nc.dram_tensor("w_outT", [DM, DM], bf16, kind="ExternalInput")
    band_d = nc.dram_tensor("band", [128, BAND_TOTAL], bf16, kind="ExternalInput")
    vdiag_d = nc.dram_tensor("vdiag", [DH, H * R * DH], bf16, kind="ExternalInput")
    gdiag_d = nc.dram_tensor("gdiag", [DH, GD_TOTAL], bf16, kind="ExternalInput")
    gvec_d = nc.dram_tensor("gvec", [DH, H * 21], f32, kind="ExternalInput")
    b_in_d = nc.dram_tensor("b_in", [128, H], f32, kind="ExternalInput")
    if has_bias:
        b_out_d = nc.dram_tensor("b_out", [1, DM], bf16, kind="ExternalInput")
    out_d = nc.dram_tensor("out", [TPC, DM], f32, kind="ExternalOutput")

    with tile.TileContext(nc) as tc:
        with tc.tile_pool(name="const", bufs=1) as pc:
            xp_sb = [pc.tile([DH, XF], bf16, name=f"xp{m}") for m in range(H)]
            o_sb = [pc.tile([DH, TPC], bf16, name=f"o{h}") for h in range(H)]
            w_sb = [pc.tile([128, DM], bf16, name=f"w_in{i}") for i in range(H)]
            xT_sb = [pc.tile([128, TH], bf16, name=f"xT{i}") for i in range(H)]
            wo_sb = [pc.tile([128, DM], bf16, name=f"w_out{i}") for i in range(H)]
            gvec_sb = pc.tile([DH, H * 21], f32, name="gvec_sb")
            vd_sb = pc.tile([DH, H * R * DH], bf16, name="vd_sb")
            gd_sb = pc.tile([DH, GD_TOTAL], bf16, name="gd_sb")
            b_in_sb = pc.tile([128, H], f32, name="b_in_sb")
            if has_bias:
                ones_sb = pc.tile([1, 128], bf16, name="ones_sb")
                bo_sb = pc.tile([1, DM], bf16, name="bo_sb")
                nc.gpsimd.memset(ones_sb, 1.0)
                nc.sync.dma_start(out=bo_sb, in_=b_out_d[:, :])

            for m in range(H):
                nc.gpsimd.memset(xp_sb[m][:, 0 : LP - PAD], 0)
                nc.gpsimd.memset(xp_sb[m][:, LP + TPC + PAD : XF], 0)

            # warm the activation table off the critical path
            warm = pc.tile([1, 2], f32, name="warm")
            nc.gpsimd.memset(warm, 0)
            nc.scalar.activation(
                out=warm[:, 1:2], in_=warm[:, 0:1], func=IDENT, bias=0.0, scale=1.0
            )

            # ---- preamble DMAs, first-needed first, spread over SP/Act ----
            m0 = HEADS[0]
            for i in range(H):
                eng = nc.sync if i % 2 == 0 else nc.scalar
                eng.dma_start(
                    out=w_sb[i][:, m0 * 128 : (m0 + 1) * 128],
                    in_=w_inT_d[i * 128 : (i + 1) * 128, m0 * 128 : (m0 + 1) * 128],
                )
                eng = nc.scalar if i % 2 == 0 else nc.sync
                eng.dma_start(
                    out=xT_sb[i][:, 0:512], in_=xT_d[i * 128 : (i + 1) * 128, 0:512]
                )
            nc.sync.dma_start(out=b_in_sb, in_=b_in_d[:, :])
            for i in range(H):
                eng = nc.scalar if i % 2 == 0 else nc.sync
                eng.dma_start(
                    out=xT_sb[i][:, 512:TH], in_=xT_d[i * 128 : (i + 1) * 128, 512:TH]
                )

            with tc.tile_pool(name="ps1", bufs=2, space="PSUM") as pp1, tc.tile_pool(
                name="ps3", bufs=2, space="PSUM"
            ) as pp3, tc.tile_pool(
                name="pso", bufs=2, space="PSUM"
            ) as pp_o, tc.tile_pool(name="pband", bufs=3) as p_band, tc.tile_pool(
                name="pcs", bufs=2
            ) as p_cs, tc.tile_pool(name="pxtd", bufs=3) as p_xtd, tc.tile_pool(
                name="pchain", bufs=2
            ) as p_ch, tc.tile_pool(name="pts", bufs=1) as p_ts:
                band_tiles = {}
                cs_tiles = {}
                xtd_tiles = {}
                chain_tiles = {}

                def issue_w_cols(m, eng):
                    for i in range(H):
                        eng.dma_start(
                            out=w_sb[i][:, m * 128 : (m + 1) * 128],
                            in_=w_inT_d[
                                i * 128 : (i + 1) * 128, m * 128 : (m + 1) * 128
                            ],
                        )

                def issue_band_dma(h):
                    boff = _band_off(h)
                    bw = NT * _tile_cols(h)
                    bt = p_band.tile([128, bw], bf16, name=f"band{h}", tag="band")
                    nc.sync.dma_start(out=bt, in_=band_d[:, boff : boff + bw])
                    band_tiles[h] = bt

                def s1_chunk(m, ci):
                    c0, cn = S1CH[ci]
                    ps1 = pp1.tile([128, 512], f32, name="ps1", tag="ps1")
                    for i in range(H):
                        nc.tensor.matmul(
                            ps1[:, :cn],
                            w_sb[i][:, m * 128 : (m + 1) * 128],
                            xT_sb[i][:, c0 : c0 + cn],
                            start=(i == 0),
                            stop=(i == H - 1),
                        )
                    nc.scalar.activation(
                        out=xp_sb[m][:, LP - PAD + c0 : LP - PAD + c0 + cn],
                        in_=ps1[:, :cn],
                        func=IDENT,
                        bias=b_in_sb[:, m : m + 1],
                        scale=1.0,
                    )

                def issue_transpose(m):
                    xtd = p_xtd.tile([128, XF // 128, 128], bf16, name="xtd", tag="xtd")
                    nc.sync.dma_start_transpose(out=xtd, in_=xp_sb[m])
                    xtd_tiles[m] = xtd

                def chain(eng, tile_out, taps):
                    in0, sc = taps[0]
                    eng.tensor_scalar(
                        out=tile_out, in0=in0, scalar1=sc, scalar2=None, op0=MULT
                    )
                    for in0, sc in taps[1:]:
                        eng.scalar_tensor_tensor(
                            out=tile_out,
                            in0=in0,
                            scalar=sc,
                            in1=tile_out,
                            op0=MULT,
                            op1=ADD,
                        )

                def band_pair(h, pair):
                    """Band matmuls for token tiles 2*pair, 2*pair+1 into one
                    2-bank PSUM tile + a single paired Act evac."""
                    k = KS[h]
                    p = k // 2
                    tcols = _tile_cols(h)
                    bt = band_tiles[h]
                    xtd = xtd_tiles[h]
                    if pair == 0:
                        cs = p_cs.tile([128, R, TPC], bf16, name=f"cs{h}", tag="cs")
                        cs_tiles[h] = cs
                    cs = cs_tiles[h]
                    psp = pp3.tile([128, 2, R, 128], f32, name="ps_s", tag="ps_s")
                    for half in range(2):
                        b = 2 * pair + half
                        o = b * tcols
                        bC = bt[:, o : o + R * 128].rearrange(
                            "q (r w) -> q r w", r=R
                        )
                        bL = bt[:, o + R * 128 : o + R * 128 + R * p].rearrange(
                            "q (r w) -> q r w", r=R
                        )
                        bR = bt[:, o + R * 128 + R * p : o + tcols].rearrange(
                            "q (r w) -> q r w", r=R
                        )
                        ps_s = psp[:, half, :, :]
                        nc.tensor.matmul(
                            ps_s, xtd[:, b + 1, :], bC, start=True, stop=False
                        )
                        nc.tensor.matmul(
                            ps_s[:, :, 0:p], xtd[:, b, :], bL, start=False, stop=False
                        )
                        nc.tensor.matmul(
                            ps_s[:, :, 128 - p : 128],
                            xtd[:, b + 2, :],
                            bR,
                            start=False,
                            stop=True,
                        )
                    b0 = 2 * pair
                    nc.scalar.copy(
                        cs[:, :, b0 * 128 : (b0 + 2) * 128],
                        psp.rearrange("q b r w -> q r b w"),
                    )
                    if pair == 3:
                        band_tiles.pop(h)
                        xtd_tiles.pop(h)

                def band_chains(h):
                    # DVE static MAC chains for taps not handled on PE;
                    # overlap with next group's PE work; merged in tail_stage
                    k = KS[h]
                    p = k // 2
                    j0 = PE_TAPS.get(h, 0)
                    if j0 < k:
                        gv = gvec_sb
                        taps = [
                            (
                                xp_sb[h][:, LP + j - p : LP + j - p + TPC],
                                gv[:, h * 21 + j : h * 21 + j + 1],
                            )
                            for j in range(j0, k)
                        ]
                        tiles = []
                        for ci in range(0, len(taps), 7):
                            ct = p_ch.tile(
                                [DH, TPC], bf16, name=f"ch{h}", tag=f"ch{ci // 7}"
                            )
                            chain(nc.vector, ct, taps[ci : ci + 7])
                            tiles.append(ct)
                        while len(tiles) > 1:
                            nc.vector.tensor_add(tiles[0], tiles[0], tiles[1])
                            tiles = [tiles[0]] + tiles[2:]
                        chain_tiles[h] = tiles[0]

                def tail_stage(h):
                    cs = cs_tiles.pop(h)
                    k = KS[h]
                    p = k // 2
                    j0 = PE_TAPS.get(h, 0)
                    has_dve = h in chain_tiles
                    tmp_o = None
                    if has_dve:
                        tmp_o = p_ch.tile([DH, TPC], bf16, name="tmp_o", tag="tmpo")
                    for ci, c0 in enumerate((0, 512)):
                        ps_o = pp_o.tile([128, 512], f32, name="ps_o", tag="ps_o")
                        n_mm = R + j0
                        idx = 0
                        for r in range(R):
                            nc.tensor.matmul(
                                ps_o,
                                vd_sb[:, (h * R + r) * DH : (h * R + r + 1) * DH],
                                cs[:, r, c0 : c0 + 512],
                                start=(idx == 0),
                                stop=(idx == n_mm - 1),
                            )
                            idx += 1
                        go = GD_OFF.get(h, 0)
                        for j in range(j0):
                            nc.tensor.matmul(
                                ps_o,
                                gd_sb[:, go + j * DH : go + (j + 1) * DH],
                                xp_sb[h][:, LP + j - p + c0 : LP + j - p + c0 + 512],
                                start=False,
                                stop=(idx == n_mm - 1),
                            )
                            idx += 1
                        # fast Act evac so the PSUM bank frees without
                        # waiting on the DVE chain backlog
                        dst = tmp_o if has_dve else o_sb[h]
                        nc.scalar.copy(dst[:, c0 : c0 + 512], ps_o)
                    if has_dve:
                        sacc = chain_tiles.pop(h)
                        nc.vector.tensor_add(o_sb[h], tmp_o, sacc)

                # ---------------- pipelined emission ----------------
                # PE warm-up on zeroed data so the pstate ramp completes
                # while the first input DMAs land
                wscr = pc.tile([128, 512], bf16, name="wscr")
                nc.vector.memset(wscr, 0)
                for _ in range(6):
                    psw = pp1.tile([128, 512], f32, name="ps1", tag="ps1")
                    nc.tensor.matmul(psw, wscr[:, 0:128], wscr, start=True, stop=True)

                issue_w_cols(HEADS[1], nc.scalar)
                issue_band_dma(HEADS[0])
                # rest of w_in (cols 0:768 — m0, m1 are blocks 6 and 7)
                for i in range(H):
                    eng = nc.sync if i % 2 == 0 else nc.scalar
                    eng.dma_start(
                        out=w_sb[i][:, 0 : 6 * 128],
                        in_=w_inT_d[i * 128 : (i + 1) * 128, 0 : 6 * 128],
                    )
                nc.sync.dma_start(out=gvec_sb, in_=gvec_d[:, :])
                nc.sync.dma_start(out=vd_sb, in_=vdiag_d[:, :])
                nc.sync.dma_start(out=gd_sb, in_=gdiag_d[:, :])
                issue_band_dma(HEADS[1])
                for gi, m in enumerate(HEADS):
                    hp = HEADS[gi - 2] if gi >= 2 else None  # band stage
                    hq = HEADS[gi - 3] if gi >= 3 else None  # tail stage
                    if gi >= 2:
                        issue_band_dma(HEADS[gi])
                    if gi == 3:
                        for i in range(H):
                            nc.sync.dma_start(
                                out=wo_sb[i],
                                in_=w_outT_d[i * 128 : (i + 1) * 128, :],
                            )
                    s1_chunk(m, 0)
                    if hp is not None:
                        band_pair(hp, 0)
                    s1_chunk(m, 1)
                    if hp is not None:
                        band_pair(hp, 1)
                    s1_chunk(m, 2)
                    issue_transpose(m)
                    if hp is not None:
                        band_pair(hp, 2)
                        band_pair(hp, 3)
                    if hq is not None:
                        tail_stage(hq)
                    if hp is not None:
                        band_chains(hp)
                for hl in (HEADS[6], HEADS[7]):
                    for pair in range(4):
                        band_pair(hl, pair)
                    tail_stage(HEADS[HEADS.index(hl) - 1])
                    band_chains(hl)
                tail_stage(HEADS[7])

            # ---------------- stage 4: out projection ----------------
            with tc.tile_pool(name="ps4", bufs=4, space="PSUM") as pp4, tc.tile_pool(
                name="post", bufs=4
            ) as p_ost:
                for t in range(NT):
                    for ei, e0 in enumerate((0, 512)):
                        ps4 = pp4.tile([128, 512], f32, name="ps4", tag="ps4")
                        n_mm = H + (1 if has_bias else 0)
                        for i in range(H):
                            nc.tensor.matmul(
                                ps4,
                                o_sb[i][:, t * 128 : (t + 1) * 128],
                                wo_sb[i][:, e0 : e0 + 512],
                                start=(i == 0),
                                stop=(i == n_mm - 1),
                            )
                        if has_bias:
                            nc.tensor.matmul(
                                ps4,
                                ones_sb,
                                bo_sb[:, e0 : e0 + 512],
                                start=False,
                                stop=True,
                            )
                        ost = p_ost.tile([128, 512], f32, name="ost", tag="ost")
                        nc.vector.tensor_scalar(
                            out=ost, in0=ps4, scalar1=1.0, scalar2=None, op0=MULT
                        )
                        eng = nc.sync if ei == 0 else nc.scalar
                        eng.dma_start(
                            out=out_d[t * 128 : (t + 1) * 128, e0 : e0 + 512],
                            in_=ost,
                        )

    _split_multi_waits(nc, mybir)
    return nc


def _band_bases(A):
    """Per-head unscaled band blocks (f32): C (128,R,128), L/R (128,R,p)."""
    bases = []
    t = np.arange(128)[:, None]
    for h in range(H):
        k = KS[h]
        p = k // 2
        w = np.arange(128)[None, :]
        dC = t - w
        mC = np.abs(dC) <= p
        iC = np.clip(dC + p, 0, k - 1)
        wl = np.arange(p)[None, :] if p else np.zeros((1, 0), int)
        dL = t - wl - 128
        mL = (dL >= -p) & (dL <= p)
        iL = np.clip(dL + p, 0, k - 1)
        u = np.arange(p)[None, :] if p else np.zeros((1, 0), int)
        dR = t + p - u  # t - (128-p+u) + 128
        mR = (dR >= -p) & (dR <= p)
        iR = np.clip(dR + p, 0, k - 1)
        C = np.where(mC[:, None, :], A[h][:, iC].transpose(1, 0, 2), 0.0)
        L = np.where(mL[:, None, :], A[h][:, iL].transpose(1, 0, 2), 0.0)
        Rb = np.where(mR[:, None, :], A[h][:, iR].transpose(1, 0, 2), 0.0)
        bases.append((C, L, Rb))
    return bases


def _host_prep(inputs):
    x = np.ascontiguousarray(np.asarray(inputs["x"], dtype=np.float32))
    W_in = np.asarray(inputs["W_in"], dtype=np.float32)
    b_in = np.asarray(inputs["b_in"], dtype=np.float32)
    W_out = np.asarray(inputs["W_out"], dtype=np.float32)
    b_out = np.asarray(inputs["b_out"], dtype=np.float32)
    Wc = np.asarray(inputs["Wc"], dtype=np.float32)
    A = np.asarray(inputs["A"], dtype=np.float32)
    V = np.asarray(inputs["V"], dtype=np.float32)
    base = np.asarray(inputs["base"], dtype=np.float32)
    alphas = np.asarray(inputs["alphas"], dtype=np.float32)

    alpha = 1.0 / (1.0 + np.exp(-alphas))
    W_inT = np.ascontiguousarray(W_in.T)
    W_outT = np.ascontiguousarray(W_out.T)
    Wc_aug = np.zeros((DM, H * R), dtype=np.float32)
    for h in range(H):
        # alpha folded into c
        Wc_aug[:, R * h : R * h + R] = alpha[h] * (
            W_inT[:, h * DH : (h + 1) * DH] @ Wc[h]
        )

    bases = _band_bases(A)

    gvec = np.zeros((DH, H, 21), dtype=np.float32)
    for h in range(H):
        k = KS[h]
        gvec[:, h, :k] = ((1.0 - alpha[h]) * base[h, :k]).T

    dd = np.arange(DH)
    vd = np.zeros((DH, H, R, DH), dtype=np.float32)
    for h in range(H):
        for r in range(R):
            vd[dd, h, r, dd] = V[h, r]
    gd = np.zeros((DH, GD_TOTAL), dtype=np.float32)
    for h in sorted(PE_TAPS):
        go = GD_OFF[h]
        g = (1.0 - alpha[h]) * base[h, : PE_TAPS[h]]  # (j0, DH)
        for j in range(PE_TAPS[h]):
            gd[dd, go + j * DH + dd] = g[j]

    prep = {
        "w_inT": W_inT.astype(BF16),
        "w_outT": W_outT.astype(BF16),
        "vdiag": vd.reshape(DH, H * R * DH).astype(BF16),
        "gdiag": gd.astype(BF16),
        "gvec": gvec.reshape(DH, H * 21).copy(),
        "b_in": np.ascontiguousarray(b_in.reshape(H, 128).T),
    }
    has_bias = bool(np.any(b_out != 0.0))
    if has_bias:
        prep["b_out"] = b_out.reshape(1, DM).astype(BF16)

    xT_slices = []
    band_slices = []
    per_b = NC // B
    for c in range(NC):
        bb = c // per_b
        s = (c % per_b) * TPC
        sl = np.zeros((TH, DM), dtype=np.float32)
        lo, hi = s - PAD, s + TPC + PAD
        clo, chi = max(lo, 0), min(hi, N)
        sl[clo - lo : chi - lo] = x[bb, clo:chi]
        xT_slices.append(np.ascontiguousarray(sl.T).astype(BF16))
        cc = (sl[PAD : PAD + TPC] @ Wc_aug).T.reshape(H, R, TPC)  # alpha*c

        band = np.empty((128, BAND_TOTAL), dtype=np.float32)
        for h in range(H):
            k = KS[h]
            p = k // 2
            C, L, Rb = bases[h]
            tcols = _tile_cols(h)
            boff = _band_off(h)
            ch = cc[h]  # (R, TPC)
            for b in range(NT):
                o = boff + b * tcols
                cw = ch[None, :, b * 128 : (b + 1) * 128]  # (1, R, 128)
                band[:, o : o + R * 128] = (C * cw).reshape(128, R * 128)
                if p:
                    cl = ch[None, :, b * 128 : b * 128 + p]
                    band[:, o + R * 128 : o + R * 128 + R * p] = (L * cl).reshape(
                        128, R * p
                    )
                    cr = ch[None, :, (b + 1) * 128 - p : (b + 1) * 128]
                    band[:, o + R * 128 + R * p : o + tcols] = (Rb * cr).reshape(
                        128, R * p
                    )
        band_slices.append(band.astype(BF16))
    return prep, xT_slices, band_slices, has_bias


def _run(inputs, trace=False, **kwargs):
    _install_ntff_hook_shim()
    from concourse.bass_utils import run_bass_kernel_spmd

    prep, xT_slices, band_slices, has_bias = _host_prep(inputs)
    key = ("mod", has_bias)
    if key not in _MODULE_CACHE:
        _MODULE_CACHE[key] = _build_module(has_bias)
    nc = _MODULE_CACHE[key]

    in_maps = []
    for c in range(NC):
        m = dict(prep)
        m["xT"] = xT_slices[c]
        m["band"] = band_slices[c]
        in_maps.append(m)

    res = run_bass_kernel_spmd(
        nc, in_maps, core_ids=list(range(NC)), trace=trace, **kwargs
    )
    outs = [res.results[c]["out"] for c in range(NC)]
    full = np.concatenate(outs, axis=0).reshape(B, N, DM).astype(np.float32)
    return full, res


def kernel(**inputs) -> np.ndarray:
    return _run(inputs)[0]


# revision 44
# speedup vs baseline: 1.0459x; 1.0459x over previous
"""Trainium2 Bass kernel for nn_DKAModule (dynamic-kernel attention), v3.

Data-parallel over B*n = 8192 tokens -> TPC=1024 per core (+10-token halo).
All matmuls bf16 (1 cycle/col on PE at 2.4GHz when back-to-back).

Per core, software-pipelined over heads (group g runs stage1 of head g,
band matmuls of head g-1, diag/chain tail of head g-2):

  stage1:  xp_m = W_in-block^T @ x^T + b_in     (PE; Act evac to bf16)
  band:    xtd  = 10-tile DMA transpose of xp_h (1 trigger, SP)
           ps_s = per-128-token-tile banded-conv matmuls (PE: C + L/R
                  halo slices, accumulated in PSUM). The per-token
                  coefficients c (= alpha * x_proj_h @ Wc) are folded
                  into the band matrices ON HOST, so ps_s = S*c already.
           cs   = plain PSUM evac (Act copy, bf16)
  tail:    o_h  = sum_r diag(V_r) @ cs_r        (PE, vdiag)
                + static conv:  k=11 heads via diag matmuls (PE, gdiag)
                                k=3,7,21 heads via DVE STT MAC chains
                  (chains <= 7 long for bf16 accumulation error)
  stage4:  out  = o^T-blocks @ W_out^T (+ b_out) (PE; Act/DVE evac)
"""
import sys
import types

import ml_dtypes
import numpy as np

BF16 = ml_dtypes.bfloat16

KS = [3, 3, 7, 7, 11, 11, 21, 21]
H, DM, DH, R, B, N = 8, 1024, 128, 4, 2, 4096
NC = 8
TPC = B * N // NC
PAD = 10
LP = 128  # left zero-pad columns in xp
XF = 1280  # padded xp width = 10 transpose tiles
NT = TPC // 128  # 8 token tiles
TH = TPC + 2 * PAD  # 1044 valid x columns
HEADS = (6, 7, 4, 5, 2, 3, 1, 0)  # k=21 first (long DVE chains), k=3 last
PE_STATIC_HEADS = (4, 5)  # k=11 static conv via PE diag matmuls
S1CH = [(0, 512), (512, 512), (1024, 20)]

_MODULE_CACHE = {}


def _install_ntff_hook_shim():
    """This image's antenv lacks axon_hooks; provide it so profiling works."""
    if "antenv.axon_hooks" in sys.modules:
        return
    try:
        from trn_agent_boot.trn_boot import _ntff_profile_via_ctypes

        hook = _ntff_profile_via_ctypes("/opt/axon/libaxon_pjrt.so")
    except Exception:
        hook = None
    mod = types.ModuleType("antenv.axon_hooks")
    mod.get_axon_ntff_profile_hook = lambda: hook
    mod.set_axon_ntff_profile_hook = lambda h: None
    sys.modules["antenv.axon_hooks"] = mod


def _split_multi_waits(nc, mybir):
    """walrus codegen allows a single sync-wait per instruction; hoist
    extras onto a chain of single-wait NoOps on the same engine."""
    for f in nc.m.functions:
        for blk in f.blocks:
            new_insts = []
            for inst in blk.instructions:
                si = getattr(inst, "sync_info", None)
                ow = list(si.on_wait) if si and si.on_wait else []
                if len(ow) >= 2:
                    for i, w in enumerate(ow[:-1]):
                        new_insts.append(
                            mybir.InstNoOp(
                                name=f"{inst.name}-wn{i}",
                                ins=[],
                                outs=[],
                                engine=inst.engine,
                                sync_info=mybir.SyncInfo(on_wait=[w], on_update=[]),
                            )
                        )
                    inst.sync_info = mybir.SyncInfo(
                        on_wait=[ow[-1]],
                        on_update=list(si.on_update) if si.on_update else [],
                    )
                new_insts.append(inst)
            blk.instructions = new_insts


def _tile_cols(h):
    """Band cols per token-tile for head h: C (R*128) + L (R*p) + R (R*p)."""
    p = KS[h] // 2
    return R * (128 + 2 * p)


def _band_off(h):
    """Column offset of head h's packed per-tile band blocks."""
    off = 0
    for g in range(h):
        off += NT * _tile_cols(g)
    return off


BAND_TOTAL = _band_off(H - 1) + NT * _tile_cols(H - 1)

# static taps handled on PE via gdiag matmuls (rest go to DVE chains)
PE_TAPS = {4: 11, 5: 11}
GD_OFF = {}
_o = 0
for _h in sorted(PE_TAPS):
    GD_OFF[_h] = _o
    _o += PE_TAPS[_h] * DH
GD_TOTAL = _o


def _build_module(has_bias):
    import concourse.bass as bass
    import concourse.tile as tile
    from concourse import mybir

    f32 = mybir.dt.float32
    bf16 = mybir.dt.bfloat16
    MULT = mybir.AluOpType.mult
    ADD = mybir.AluOpType.add
    IDENT = mybir.ActivationFunctionType.Identity

    nc = bass.Bass(trn_type="TRN2")

    xT_d = nc.dram_tensor("xT", [DM, TH], bf16, kind="ExternalInput")
    w_inT_d = nc.dram_tensor("w_inT", [DM, DM], bf16, kind="ExternalInput")
    w_outT_d = nc.dram_tensor("w_outT", [DM, DM], bf16, kind="ExternalInput")
    band_d = nc.dram_tensor("band", [128, BAND_TOTAL], bf16, kind="ExternalInput")
    vdiag_d = nc.dram_tensor("vdiag", [DH, H * R * DH], bf16, kind="ExternalInput")
    gdiag_d = nc.dram_tensor("gdiag", [DH, GD_TOTAL], bf16, kind="ExternalInput")
    gvec_d = nc.dram_tensor("gvec", [DH, H * 21], f32, kind="ExternalInput")
    b_in_d = nc.dram_tensor("b_in", [128, H], f32, kind="ExternalInput")
    if has_bias:
        b_out_d = nc.dram_tensor("b_out", [1, DM], bf16, kind="ExternalInput")
    out_d = nc.dram_tensor("out", [TPC, DM], f32, kind="ExternalOutput")

    with tile.TileContext(nc) as tc:
        with tc.tile_pool(name="const", bufs=1) as pc:
            xp_sb = [pc.tile([DH, XF], bf16, name=f"xp{m}") for m in range(H)]
            o_sb = [pc.tile([DH, TPC], bf16, name=f"o{h}") for h in range(H)]
            w_sb = [pc.tile([128, DM], bf16, name=f"w_in{i}") for i in range(H)]
            xT_sb = [pc.tile([128, TH], bf16, name=f"xT{i}") for i in range(H)]
            wo_sb = [pc.tile([128, DM], bf16, name=f"w_out{i}") for i in range(H)]
            gvec_sb = pc.tile([DH, H * 21], f32, name="gvec_sb")
            vd_sb = pc.tile([DH, H * R * DH], bf16, name="vd_sb")
            gd_sb = pc.tile([DH, GD_TOTAL], bf16, name="gd_sb")
            b_in_sb = pc.tile([128, H], f32, name="b_in_sb")
            if has_bias:
                ones_sb = pc.tile([1, 128], bf16, name="ones_sb")
                bo_sb = pc.tile([1, DM], bf16, name="bo_sb")
                nc.gpsimd.memset(ones_sb, 1.0)
                nc.sync.dma_start(out=bo_sb, in_=b_out_d[:, :])

            for m in range(H):
                nc.gpsimd.memset(xp_sb[m][:, 0 : LP - PAD], 0)
                nc.gpsimd.memset(xp_sb[m][:, LP + TPC + PAD : XF], 0)

            # warm the activation table off the critical path
            warm = pc.tile([1, 2], f32, name="warm")
            nc.gpsimd.memset(warm, 0)
            nc.scalar.activation(
                out=warm[:, 1:2], in_=warm[:, 0:1], func=IDENT, bias=0.0, scale=1.0
            )

            # ---- preamble DMAs, first-needed first, spread over SP/Act ----
            m0 = HEADS[0]
            for i in range(H):
                eng = nc.sync if i % 2 == 0 else nc.scalar
                eng.dma_start(
                    out=w_sb[i][:, m0 * 128 : (m0 + 1) * 128],
                    in_=w_inT_d[i * 128 : (i + 1) * 128, m0 * 128 : (m0 + 1) * 128],
                )
                eng = nc.scalar if i % 2 == 0 else nc.sync
                eng.dma_start(
                    out=xT_sb[i][:, 0:512], in_=xT_d[i * 128 : (i + 1) * 128, 0:512]
                )
            nc.sync.dma_start(out=b_in_sb, in_=b_in_d[:, :])
            for i in range(H):
                eng = nc.scalar if i % 2 == 0 else nc.sync
                eng.dma_start(
                    out=xT_sb[i][:, 512:TH], in_=xT_d[i * 128 : (i + 1) * 128, 512:TH]
                )

            with tc.tile_pool(name="ps1", bufs=2, space="PSUM") as pp1, tc.tile_pool(
                name="ps3", bufs=2, space="PSUM"
            ) as pp3, tc.tile_pool(
                name="pso", bufs=2, space="PSUM"
            ) as pp_o, tc.tile_pool(name="pband", bufs=3) as p_band, tc.tile_pool(
                name="pcs", bufs=2
            ) as p_cs, tc.tile_pool(name="pxtd", bufs=3) as p_xtd, tc.tile_pool(
                name="pchain", bufs=2
            ) as p_ch, tc.tile_pool(name="pts", bufs=1) as p_ts:
                band_tiles = {}
                cs_tiles = {}
                xtd_tiles = {}
                chain_tiles = {}

                def issue_w_cols(m, eng):
                    for i in range(H):
                        eng.dma_start(
                            out=w_sb[i][:, m * 128 : (m + 1) * 128],
                            in_=w_inT_d[
                                i * 128 : (i + 1) * 128, m * 128 : (m + 1) * 128
                            ],
                        )

                def issue_band_dma(h):
                    boff = _band_off(h)
                    bw = NT * _tile_cols(h)
                    bt = p_band.tile([128, bw], bf16, name=f"band{h}", tag="band")
                    nc.sync.dma_start(out=bt, in_=band_d[:, boff : boff + bw])
                    band_tiles[h] = bt

                def s1_chunk(m, ci):
                    c0, cn = S1CH[ci]
                    ps1 = pp1.tile([128, 512], f32, name="ps1", tag="ps1")
                    for i in range(H):
                        nc.tensor.matmul(
                            ps1[:, :cn],
                            w_sb[i][:, m * 128 : (m + 1) * 128],
                            xT_sb[i][:, c0 : c0 + cn],
                            start=(i == 0),
                            stop=(i == H - 1),
                        )
                    nc.scalar.activation(
                        out=xp_sb[m][:, LP - PAD + c0 : LP - PAD + c0 + cn],
                        in_=ps1[:, :cn],
                        func=IDENT,
                        bias=b_in_sb[:, m : m + 1],
                        scale=1.0,
                    )

                def issue_transpose(m):
                    xtd = p_xtd.tile([128, XF // 128, 128], bf16, name="xtd", tag="xtd")
                    nc.sync.dma_start_transpose(out=xtd, in_=xp_sb[m])
                    xtd_tiles[m] = xtd

                def chain(eng, tile_out, taps):
                    in0, sc = taps[0]
                    eng.tensor_scalar(
                        out=tile_out, in0=in0, scalar1=sc, scalar2=None, op0=MULT
                    )
                    for in0, sc in taps[1:]:
                        eng.scalar_tensor_tensor(
                            out=tile_out,
                            in0=in0,
                            scalar=sc,
                            in1=tile_out,
                            op0=MULT,
                            op1=ADD,
                        )

                def band_pair(h, pair):
                    """Band matmuls for token tiles 2*pair, 2*pair+1 into one
                    2-bank PSUM tile + a single paired Act evac."""
                    k = KS[h]
                    p = k // 2
                    tcols = _tile_cols(h)
                    bt = band_tiles[h]
                    xtd = xtd_tiles[h]
                    if pair == 0:
                        cs = p_cs.tile([128, R, TPC], bf16, name=f"cs{h}", tag="cs")
                        cs_tiles[h] = cs
                    cs = cs_tiles[h]
                    psp = pp3.tile([128, 2, R, 128], f32, name="ps_s", tag="ps_s")
                    for half in range(2):
                        b = 2 * pair + half
                        o = b * tcols
                        bC = bt[:, o : o + R * 128].rearrange(
                            "q (r w) -> q r w", r=R
                        )
                        bL = bt[:, o + R * 128 : o + R * 128 + R * p].rearrange(
                            "q (r w) -> q r w", r=R
                        )
                        bR = bt[:, o + R * 128 + R * p : o + tcols].rearrange(
                            "q (r w) -> q r w", r=R
                        )
                        ps_s = psp[:, half, :, :]
                        nc.tensor.matmul(
                            ps_s, xtd[:, b + 1, :], bC, start=True, stop=False
                        )
                        nc.tensor.matmul(
                            ps_s[:, :, 0:p], xtd[:, b, :], bL, start=False, stop=False
                        )
                        nc.tensor.matmul(
                            ps_s[:, :, 128 - p : 128],
                            xtd[:, b + 2, :],
                            bR,
                            start=False,
                            stop=True,
                        )
                    b0 = 2 * pair
                    nc.scalar.copy(
                        cs[:, :, b0 * 128 : (b0 + 2) * 128],
                        psp.rearrange("q b r w -> q r b w"),
                    )
                    if pair == 3:
                        band_tiles.pop(h)
                        xtd_tiles.pop(h)

                def band_chains(h):
                    # DVE static MAC chains for taps not handled on PE;
                    # overlap with next group's PE work; merged in tail_stage
                    k = KS[h]
                    p = k // 2
                    j0 = PE_TAPS.get(h, 0)
                    if j0 < k:
                        gv = gvec_sb
                        taps = [
                            (
                                xp_sb[h][:, LP + j - p : LP + j - p + TPC],
                                gv[:, h * 21 + j : h * 21 + j + 1],
                            )
                            for j in range(j0, k)
                        ]
                        tiles = []
                        for ci in range(0, len(taps), 7):
                            ct = p_ch.tile(
                                [DH, TPC], bf16, name=f"ch{h}", tag=f"ch{ci // 7}"
                            )
                            chain(nc.vector, ct, taps[ci : ci + 7])
                            tiles.append(ct)
                        while len(tiles) > 1:
                            nc.vector.tensor_add(tiles[0], tiles[0], tiles[1])
                            tiles = [tiles[0]] + tiles[2:]
                        chain_tiles[h] = tiles[0]

                def tail_stage(h):
                    cs = cs_tiles.pop(h)
                    k = KS[h]
                    p = k // 2
                    j0 = PE_TAPS.get(h, 0)
                    has_dve = h in chain_tiles
                    tmp_o = None
                    if has_dve:
                        tmp_o = p_ch.tile([DH, TPC], bf16, name="tmp_o", tag="tmpo")
                    for ci, c0 in enumerate((0, 512)):
                        ps_o = pp_o.tile([128, 512], f32, name="ps_o", tag="ps_o")
                        n_mm = R + j0
                        idx = 0
                        for r in range(R):
                            nc.tensor.matmul(
                                ps_o,
                                vd_sb[:, (h * R + r) * DH : (h * R + r + 1) * DH],
                                cs[:, r, c0 : c0 + 512],
                                start=(idx == 0),
                                stop=(idx == n_mm - 1),
                            )
                            idx += 1
                        go = GD_OFF.get(h, 0)
                        for j in range(j0):
                            nc.tensor.matmul(
                                ps_o,
                                gd_sb[:, go + j * DH : go + (j + 1) * DH],
                                xp_sb[h][:, LP + j - p + c0 : LP + j - p + c0 + 512],
                                start=False,
                                stop=(idx == n_mm - 1),
                            )
                            idx += 1
                        # fast Act evac so the PSUM bank frees without
                        # waiting on the DVE chain backlog
                        dst = tmp_o if has_dve else o_sb[h]
                        nc.scalar.copy(dst[:, c0 : c0 + 512], ps_o)
                    if has_dve:
                        sacc = chain_tiles.pop(h)
                        nc.vector.tensor_add(o_sb[h], tmp_o, sacc)

                # ---------------- pipelined emission ----------------
                # PE warm-up on zeroed data so the pstate ramp completes
                # while the first input DMAs land
                wscr = pc.tile([128, 512], bf16, name="wscr")
                nc.vector.memset(wscr, 0)
                for _ in range(6):
                    psw = pp1.tile([128, 512], f32, name="ps1", tag="ps1")
                    nc.tensor.matmul(psw, wscr[:, 0:128], wscr, start=True, stop=True)

                issue_w_cols(HEADS[1], nc.scalar)
                issue_band_dma(HEADS[0])
                # rest of w_in (cols 0:768 — m0, m1 are blocks 6 and 7)
                for i in range(H):
                    eng = nc.sync if i % 2 == 0 else nc.scalar
                    eng.dma_start(
                        out=w_sb[i][:, 0 : 6 * 128],
                        in_=w_inT_d[i * 128 : (i + 1) * 128, 0 : 6 * 128],
                    )
                nc.sync.dma_start(out=gvec_sb, in_=gvec_d[:, :])
                nc.sync.dma_start(out=vd_sb, in_=vdiag_d[:, :])
                nc.sync.dma_start(out=gd_sb, in_=gdiag_d[:, :])
                issue_band_dma(HEADS[1])
                for gi, m in enumerate(HEADS):
                    hp = HEADS[gi - 1] if gi >= 1 else None  # band stage
                    hq = HEADS[gi - 2] if gi >= 2 else None  # tail stage
                    if gi + 2 < H:
                        issue_band_dma(HEADS[gi + 2])
                    if gi == 3:
                        for i in range(H):
                            nc.sync.dma_start(
                                out=wo_sb[i],
                                in_=w_outT_d[i * 128 : (i + 1) * 128, :],
                            )
                    s1_chunk(m, 0)
                    if hp is not None:
                        band_pair(hp, 0)
                    s1_chunk(m, 1)
                    if hp is not None:
                        band_pair(hp, 1)
                    s1_chunk(m, 2)
                    issue_transpose(m)
                    if hp is not None:
                        band_pair(hp, 2)
                        band_pair(hp, 3)
                    if hq is not None:
                        tail_stage(hq)
                    if hp is not None:
                        band_chains(hp)
                for pair in range(4):
                    band_pair(HEADS[7], pair)
                tail_stage(HEADS[6])
                band_chains(HEADS[7])
                tail_stage(HEADS[7])

            # ---------------- stage 4: out projection ----------------
            with tc.tile_pool(name="ps4", bufs=4, space="PSUM") as pp4, tc.tile_pool(
                name="post", bufs=4
            ) as p_ost:
                for t in range(NT):
                    for ei, e0 in enumerate((0, 512)):
                        ps4 = pp4.tile([128, 512], f32, name="ps4", tag="ps4")
                        n_mm = H + (1 if has_bias else 0)
                        for i in range(H):
                            nc.tensor.matmul(
                                ps4,
                                o_sb[i][:, t * 128 : (t + 1) * 128],
                                wo_sb[i][:, e0 : e0 + 512],
                                start=(i == 0),
                                stop=(i == n_mm - 1),
                            )
                        if has_bias:
                            nc.tensor.matmul(
                                ps4,
                                ones_sb,
                                bo_sb[:, e0 : e0 + 512],
                                start=False,
                                stop=True,
                            )
                        ost = p_ost.tile([128, 512], f32, name="ost", tag="ost")
                        nc.vector.tensor_scalar(
                            out=ost, in0=ps4, scalar1=1.0, scalar2=None, op0=MULT
                        )
                        eng = nc.sync if ei == 0 else nc.scalar
                        eng.dma_start(
                            out=out_d[t * 128 : (t + 1) * 128, e0 : e0 + 512],
                            in_=ost,
                        )

    _split_multi_waits(nc, mybir)
    return nc


def _band_bases(A):
    """Per-head unscaled band blocks (f32): C (128,R,128), L/R (128,R,p)."""
    bases = []
    t = np.arange(128)[:, None]
    for h in range(H):
        k = KS[h]
        p = k // 2
        w = np.arange(128)[None, :]
        dC = t - w
        mC = np.abs(dC) <= p
        iC = np.clip(dC + p, 0, k - 1)
        wl = np.arange(p)[None, :] if p else np.zeros((1, 0), int)
        dL = t - wl - 128
        mL = (dL >= -p) & (dL <= p)
        iL = np.clip(dL + p, 0, k - 1)
        u = np.arange(p)[None, :] if p else np.zeros((1, 0), int)
        dR = t + p - u  # t - (128-p+u) + 128
        mR = (dR >= -p) & (dR <= p)
        iR = np.clip(dR + p, 0, k - 1)
        C = np.where(mC[:, None, :], A[h][:, iC].transpose(1, 0, 2), 0.0)
        L = np.where(mL[:, None, :], A[h][:, iL].transpose(1, 0, 2), 0.0)
        Rb = np.where(mR[:, None, :], A[h][:, iR].transpose(1, 0, 2), 0.0)
        bases.append((C, L, Rb))
    return bases


def _host_prep(inputs):
    x = np.ascontiguousarray(np.asarray(inputs["x"], dtype=np.float32))
    W_in = np.asarray(inputs["W_in"], dtype=np.float32)
    b_in = np.asarray(inputs["b_in"], dtype=np.float32)
    W_out = np.asarray(inputs["W_out"], dtype=np.float32)
    b_out = np.asarray(inputs["b_out"], dtype=np.float32)
    Wc = np.asarray(inputs["Wc"], dtype=np.float32)
    A = np.asarray(inputs["A"], dtype=np.float32)
    V = np.asarray(inputs["V"], dtype=np.float32)
    base = np.asarray(inputs["base"], dtype=np.float32)
    alphas = np.asarray(inputs["alphas"], dtype=np.float32)

    alpha = 1.0 / (1.0 + np.exp(-alphas))
    W_inT = np.ascontiguousarray(W_in.T)
    W_outT = np.ascontiguousarray(W_out.T)
    Wc_aug = np.zeros((DM, H * R), dtype=np.float32)
    for h in range(H):
        # alpha folded into c
        Wc_aug[:, R * h : R * h + R] = alpha[h] * (
            W_inT[:, h * DH : (h + 1) * DH] @ Wc[h]
        )

    bases = _band_bases(A)

    gvec = np.zeros((DH, H, 21), dtype=np.float32)
    for h in range(H):
        k = KS[h]
        gvec[:, h, :k] = ((1.0 - alpha[h]) * base[h, :k]).T

    dd = np.arange(DH)
    vd = np.zeros((DH, H, R, DH), dtype=np.float32)
    for h in range(H):
        for r in range(R):
            vd[dd, h, r, dd] = V[h, r]
    gd = np.zeros((DH, GD_TOTAL), dtype=np.float32)
    for h in sorted(PE_TAPS):
        go = GD_OFF[h]
        g = (1.0 - alpha[h]) * base[h, : PE_TAPS[h]]  # (j0, DH)
        for j in range(PE_TAPS[h]):
            gd[dd, go + j * DH + dd] = g[j]

    prep = {
        "w_inT": W_inT.astype(BF16),
        "w_outT": W_outT.astype(BF16),
        "vdiag": vd.reshape(DH, H * R * DH).astype(BF16),
        "gdiag": gd.astype(BF16),
        "gvec": gvec.reshape(DH, H * 21).copy(),
        "b_in": np.ascontiguousarray(b_in.reshape(H, 128).T),
    }
    has_bias = bool(np.any(b_out != 0.0))
    if has_bias:
        prep["b_out"] = b_out.reshape(1, DM).astype(BF16)

    xT_slices = []
    band_slices = []
    per_b = NC // B
    for c in range(NC):
        bb = c // per_b
        s = (c % per_b) * TPC
        sl = np.zeros((TH, DM), dtype=np.float32)
        lo, hi = s - PAD, s + TPC + PAD
        clo, chi = max(lo, 0), min(hi, N)
        sl[clo - lo : chi - lo] = x[bb, clo:chi]
        xT_slices.append(np.ascontiguousarray(sl.T).astype(BF16))
        cc = (sl[PAD : PAD + TPC] @ Wc_aug).T.reshape(H, R, TPC)  # alpha*c

        band = np.empty((128, BAND_TOTAL), dtype=np.float32)
        for h in range(H):
            k = KS[h]
            p = k // 2
            C, L, Rb = bases[h]
            tcols = _tile_cols(h)
            boff = _band_off(h)
            ch = cc[h]  # (R, TPC)
            for b in range(NT):
                o = boff + b * tcols
                cw = ch[None, :, b * 128 : (b + 1) * 128]  # (1, R, 128)
                band[:, o : o + R * 128] = (C * cw).reshape(128, R * 128)
                if p:
                    cl = ch[None, :, b * 128 : b * 128 + p]
                    band[:, o + R * 128 : o + R * 128 + R * p] = (L * cl).reshape(
                        128, R * p
                    )
                    cr = ch[None, :, (b + 1) * 128 - p : (b + 1) * 128]
                    band[:, o + R * 128 + R * p : o + tcols] = (Rb * cr).reshape(
                        128, R * p
                    )
        band_slices.append(band.astype(BF16))
    return prep, xT_slices, band_slices, has_bias


def _run(inputs, trace=False, **kwargs):
    _install_ntff_hook_shim()
    from concourse.bass_utils import run_bass_kernel_spmd

    prep, xT_slices, band_slices, has_bias = _host_prep(inputs)
    key = ("mod", has_bias)
    if key not in _MODULE_CACHE:
        _MODULE_CACHE[key] = _build_module(has_bias)
    nc = _MODULE_CACHE[key]

    in_maps = []
    for c in range(NC):
        m = dict(prep)
        m["xT"] = xT_slices[c]
        m["band"] = band_slices[c]
        in_maps.append(m)

    res = run_bass_kernel_spmd(
        nc, in_maps, core_ids=list(range(NC)), trace=trace, **kwargs
    )
    outs = [res.results[c]["out"] for c in range(NC)]
    full = np.concatenate(outs, axis=0).reshape(B, N, DM).astype(np.float32)
    return full, res


def kernel(**inputs) -> np.ndarray:
    return _run(inputs)[0]
